# revision 6
# baseline (speedup 1.0000x reference)
"""Single-head attention (B=4, S=2048, E=1024) on 8 TRN2 NeuronCores.

Sharding: core c -> (batch b = c//2, sequence-half h = c%2). Each core
computes K^T and V only for its OWN 1024-column half of the sequence,
then the pair all-gathers both (through DRAM bounce buffers) so each
core assembles the full K^T [1024f, 2048k] and V [2048k, 1024f] in
ABSOLUTE key order (gather block hh is rank hh's half -> no per-core
indexing; the SPMD program is identical across cores).

Perf structure (vs the original version, ~270us -> ~224us traced):
- PE warm-up matmuls at t=0 so the HAM clock gate (1.2->2.4 GHz after
  ~3.4us of sustained PE activity) releases before real matmuls start.
- Fine-grained first-slab input tiles + et-major K-projection rounds of
  8 PSUM banks: the K matmuls stream while the 8.4MB of inputs arrive
  (DMA rings round-robin packets across queued descriptors, so coarse
  whole-tile deps would idle the PE ~13us).
- K gathered in two ks-halves into 4 per-(rank-half, ks) kT tiles; the
  scores phase walks kt in gather-arrival order (two passes), so it
  never waits on the second gather. V path on the gpsimd ring so it is
  not queued behind K gather-backs.
- Softmax denominators via fp8e4 DoubleRow matmuls (two kt tiles per
  MM) against an fp8 copy of P; numerically neutral (sum of 2048 terms
  averages out e4m3 noise) and halves the denominator matmul count.
- Output tiles in bf16 (host upcasts) -> half the output DMA.
"""

import numpy as np
import ml_dtypes

import concourse.bass as bass
import concourse.tile as tile
from concourse import bacc, mybir
from concourse.bass_utils import run_bass_kernel_spmd

B, S, E = 4, 2048, 1024
N_CORES = 8
SQ = S // 2
P = 128
NT = 512
ET = E // P        # 8
KT = S // P        # 16
KTH = SQ // P      # 8 own-half k tiles
FP32 = mybir.dt.float32
BF16 = mybir.dt.bfloat16
FP8 = mybir.dt.float8e4
SCALE = 1.0 / np.sqrt(E).astype(np.float32)
SHIFT = -4.0
PAIRS = [[0, 1], [2, 3], [4, 5], [6, 7]]
N_WARM = 16
PS_BUFS = 8


def build_kernel(ctx, tc, io):
    nc = tc.nc
    xo, wqT, wkT, wvT, bq, bk, bv, outT = (
        io["xo"], io["wqT"], io["wkT"], io["wvT"],
        io["bq"], io["bk"], io["bv"], io["outT"],
    )

    singles = ctx.enter_context(tc.tile_pool(name="singles", bufs=1))
    results = ctx.enter_context(tc.tile_pool(name="results", bufs=1))
    # kh/vh halves are dead once bounced out to DRAM; p_sb reuses the slot.
    xp_pool = ctx.enter_context(tc.tile_pool(name="xp", bufs=1))
    outp = ctx.enter_context(tc.tile_pool(name="outp", bufs=2))
    dram = ctx.enter_context(tc.tile_pool(name="dram", bufs=1, space="DRAM"))
    ps_main = ctx.enter_context(tc.tile_pool(name="ps_main", bufs=PS_BUFS,
                                             space="PSUM"))

    # ---- ScalarE LUT warm-up + PE HAM warm-up. The HAM clock gate only
    # releases (1.2 -> 2.4 GHz) after ~3.4us of sustained PE activity, so
    # burn that window on dummy matmuls while the input DMAs stream in.
    wdum = singles.tile([P, NT], BF16)
    nc.vector.memset(wdum, 0.0)
    for _ in range(N_WARM):
        ps = ps_main.tile([P, NT], FP32, name="ps")
        nc.tensor.matmul(ps[:, 0:P], lhsT=wdum[:, 0:P], rhs=wdum[:, 0:P],
                         start=True, stop=True)
    warm = singles.tile([1, 4], FP32)
    warmb = singles.tile([1, 1], FP32)
    nc.vector.memset(warm, 0.0)
    nc.vector.memset(warmb, 0.0)
    nc.scalar.activation(out=warm[:, 0:2], in_=warm[:, 0:2],
                         func=mybir.ActivationFunctionType.Identity,
                         bias=warmb, scale=1.0)
    nc.scalar.activation(out=warm[:, 2:4], in_=warm[:, 2:4],
                         func=mybir.ActivationFunctionType.Exp,
                         bias=warmb, scale=1.0)

    # ---- input staging. Tiny bias DMAs go first on the gpsimd ring (they
    # gate the first ACTIVATEs). Slabs split across the vector/scalar/gpsimd
    # rings so arrival order matches first use and no ring ever carries a
    # blocking semaphore wait in front of input data.
    bq_sb = singles.tile([P, ET], FP32)
    bk_sb = singles.tile([P, ET], FP32)
    bv_bc = singles.tile([P, E], FP32)
    nc.scalar.dma_start(out=bk_sb, in_=bk.rearrange("(t p) -> p t", p=P))
    nc.scalar.dma_start(out=bq_sb, in_=bq.rearrange("(t p) -> p t", p=P))

    # Slab tiles. et=0 is split finely (per-ft wk pieces, per-ks xo pieces)
    # so the very first matmul group only waits on ~160KB; the DMA rings
    # round-robin packets across queued descriptors, so a coarse first slab
    # would otherwise land at the same late time as every other slab.
    WKF = 256
    wk0_f = [singles.tile([P, WKF], BF16, name=f"wk0f{f}")
             for f in range(E // WKF)]
    xo0_k = [singles.tile([P, NT], BF16, name=f"xo0k{k}")
             for k in range(SQ // NT)]
    wk_t = [None] + [singles.tile([P, E], BF16, name=f"wk{t}")
                     for t in range(1, ET)]
    xo_t = [None] + [singles.tile([P, SQ], BF16, name=f"xo{t}")
                     for t in range(1, ET)]
    wv_sb = singles.tile([P, ET, E], BF16)
    wq_sb = singles.tile([P, ET, E], BF16)
    for f in range(E // WKF):
        nc.gpsimd.dma_start(out=wk0_f[f], in_=wkT[0:P, f * WKF:(f + 1) * WKF])
    for k in range(SQ // NT):
        nc.sync.dma_start(out=xo0_k[k], in_=xo[0:P, k * NT:(k + 1) * NT])
    for t in range(1, ET):
        r = slice(t * P, (t + 1) * P)
        nc.gpsimd.dma_start(out=wk_t[t], in_=wkT[r, :])
        nc.sync.dma_start(out=xo_t[t], in_=xo[r, :])
    for t in range(ET):
        r = slice(t * P, (t + 1) * P)
        nc.gpsimd.dma_start(out=wv_sb[:, t, :], in_=wvT[r, :])
    # bv broadcast after the xo slabs but ahead of wq: late enough to stay
    # out of the K-phase window, early enough (~22us) for the V-projection
    # DVE bias-adds that start as soon as the K rounds finish
    nc.sync.dma_start(out=bv_bc, in_=bv.partition_broadcast(P))
    for t in range(ET):
        r = slice(t * P, (t + 1) * P)
        nc.sync.dma_start(out=wq_sb[:, t, :], in_=wqT[r, :])

    def wk_sl(et, ft):
        if et == 0:
            piece, off = divmod(ft * P, WKF)
            return wk0_f[piece][:, off:off + P]
        return wk_t[et][:, ft * P:(ft + 1) * P]

    def xo_sl(et, lo, width):
        if et == 0:
            k, off = divmod(lo, NT)
            assert off + width <= NT
            return xo0_k[k][:, off:off + width]
        return xo_t[et][:, lo:lo + width]

    # fp8 ones pair for the DoubleRow denominator matmuls; full 128-wide
    # stationary free dim (every output row = the denominator), matching
    # the production DoubleRow AP shape [Ki, 2, 128]
    ones8 = singles.tile([P, 2, P], FP8)
    nc.vector.memset(ones8, 1.0)
    shift_sb = singles.tile([P, 1], FP32)
    nc.vector.memset(shift_sb, SHIFT)

    qT_sb = results.tile([P, ET, SQ], BF16)
    # kT split into 4 chunks keyed by (hh, ks) gather-half so the scores
    # phase can start on gather-0 data while gather-1 is still in flight.
    # chunk c = hh*2 + ks holds absolute kt tiles hh*8 + ks*4 + (0..3).
    kT_c = [results.tile([P, ET, NT], BF16, name=f"kT{c}") for c in range(4)]

    def kT_sl(kt_abs, et):
        hh, r = divmod(kt_abs, ET)
        ks, j = divmod(r, 4)
        return kT_c[hh * 2 + ks][:, et, j * P:(j + 1) * P]
    v_sb = results.tile([P, KT, E], BF16)
    scr = xp_pool.tile([P, KT, SQ], BF16, tag="xp")  # kh: [:, 0:8, :], vh: [:, 8:16, :]
    kh_sb = scr[:, 0:ET, :]
    vh_sb = scr[:, ET:KT, :]

    bounce_k = [dram.tile([SQ, NT], BF16, name=f"bounce_k{i}", tag=f"bk{i}")
                for i in range(2)]
    gath_k = [dram.tile([S, NT], BF16, name=f"gath_k{i}", tag=f"gk{i}")
              for i in range(2)]
    bounce_v = dram.tile([SQ, E], BF16)
    gath_v = dram.tile([S, E], BF16)

    ident = mybir.ActivationFunctionType.Identity

    # ---- K^T own half [f, k']: et-major rounds of PS_BUFS outputs so the
    # matmuls start as soon as slab-pair et=0 lands instead of waiting for
    # all 16 input DMAs. Groups ordered ks-half-major so the ks=0 bounce +
    # gather fire as early as possible.
    # Sub-chunked rounds: the first two rounds complete the ks=0 half with
    # (ft6,ft7) isolated in a tiny round, so the ks=0 bounce + gather fire
    # as early as possible (the pair gathers serialize on one CC stream, so
    # the first trigger time is on the critical path to the scores phase).
    rounds = [[(ft, 0) for ft in range(ET)],
              [(ft, 1) for ft in range(ET)]]
    for ri, chunk in enumerate(rounds):
        pss = [ps_main.tile([P, NT], FP32, name="ps") for _ in chunk]
        for et in range(ET):
            for g, (ft, ks) in enumerate(chunk):
                nc.tensor.matmul(pss[g], lhsT=wk_sl(et, ft),
                                 rhs=xo_sl(et, ks * NT, NT),
                                 start=(et == 0), stop=(et == ET - 1))
        for g, (ft, ks) in enumerate(chunk):
            kr = slice(ks * NT, (ks + 1) * NT)
            nc.scalar.activation(out=kh_sb[:, ft, kr], in_=pss[g], func=ident,
                                 bias=bk_sb[:, ft:ft + 1], scale=1.0)
        if True:  # a ks half just completed: bounce+gather+back it
            ks = ri
            kr = slice(ks * NT, (ks + 1) * NT)
            for t in range(ET):
                nc.scalar.dma_start(out=bounce_k[ks][t * P:(t + 1) * P, :],
                                    in_=kh_sb[:, t, kr])
            nc.gpsimd.collective_compute(
                "AllGather", mybir.AluOpType.bypass, replica_groups=PAIRS,
                ins=[bounce_k[ks][:, :]], outs=[gath_k[ks][:, :]])
            for hh in range(2):
                for t in range(ET):
                    nc.sync.dma_start(
                        out=kT_c[hh * 2 + ks][:, t, :],
                        in_=gath_k[ks][hh * SQ + t * P: hh * SQ + (t + 1) * P, :])

    # ---- V own half  [k', f] -> bounce -> gather -> back (gpsimd ring so
    # the V path never queues behind the K gather-backs on sync)
    for kt in range(KTH):
        kr = slice(kt * P, (kt + 1) * P)
        for fs in range(E // NT):
            fr = slice(fs * NT, (fs + 1) * NT)
            ps = ps_main.tile([P, NT], FP32)
            for et in range(ET):
                nc.tensor.matmul(ps, lhsT=xo_sl(et, kt * P, P),
                                 rhs=wv_sb[:, et, fr],
                                 start=(et == 0), stop=(et == ET - 1))
            nc.vector.tensor_add(vh_sb[:, kt, fr], ps, bv_bc[:, fr])
        nc.gpsimd.dma_start(out=bounce_v[kt * P:(kt + 1) * P, :],
                            in_=vh_sb[:, kt, :])
    nc.gpsimd.collective_compute(
        "AllGather", mybir.AluOpType.bypass, replica_groups=PAIRS,
        ins=[bounce_v[:, :]], outs=[gath_v[:, :]])
    for kt in range(KT):
        nc.gpsimd.dma_start(out=v_sb[:, kt, :], in_=gath_v[kt * P:(kt + 1) * P, :])

    # ---- Q^T = Wq x_own + bq   [f, q]  (overlaps the gathers)
    for qs in range(SQ // NT):
        qr = slice(qs * NT, (qs + 1) * NT)
        for ft in range(ET):
            fr = slice(ft * P, (ft + 1) * P)
            ps = ps_main.tile([P, NT], FP32)
            for et in range(ET):
                nc.tensor.matmul(ps, lhsT=wq_sb[:, et, fr],
                                 rhs=xo_sl(et, qs * NT, NT),
                                 start=(et == 0), stop=(et == ET - 1))
            nc.scalar.activation(out=qT_sb[:, ft, qr], in_=ps, func=ident,
                                 bias=bq_sb[:, ft:ft + 1], scale=1.0)

    # ---- scores^T and P = exp(S^T * scale + shift)   [k, q]
    p_sb = xp_pool.tile([P, KT, SQ], BF16, tag="xp")
    p8_sb = results.tile([P, KT, SQ], FP8)   # fp8 copy feeds the denominators
    recip_sb = singles.tile([1, SQ], FP32)
    # Two passes over kt chunks in gather-arrival order: all q-slices
    # against the ks=0 chunks (c=0,2) first, then the ks=1 chunks (c=1,3) —
    # this gives gather-1's back-DMAs a full extra pass of runway.
    KT_A = [kt for c in (0, 2) for kt in
            range((c // 2) * ET, (c // 2) * ET + 4)]
    KT_B = [kt for c in (1, 3) for kt in
            range((c // 2) * ET + 4, (c // 2) * ET + 8)]
    for qs in range(SQ // NT):
        qr = slice(qs * NT, (qs + 1) * NT)
        for kt in KT_A:
            ps = ps_main.tile([P, NT], FP32, name="ps")
            for et in range(ET):
                nc.tensor.matmul(ps, lhsT=kT_sl(kt, et), rhs=qT_sb[:, et, qr],
                                 start=(et == 0), stop=(et == ET - 1))
            nc.scalar.activation(out=p_sb[:, kt, qr], in_=ps,
                                 func=mybir.ActivationFunctionType.Exp,
                                 bias=shift_sb[:, 0:1], scale=float(SCALE))
            nc.vector.tensor_scalar_add(p8_sb[:, kt, qr], p_sb[:, kt, qr], 0.0)
    for qs in range(SQ // NT):
        qr = slice(qs * NT, (qs + 1) * NT)
        for kt in KT_B:
            ps = ps_main.tile([P, NT], FP32, name="ps")
            for et in range(ET):
                nc.tensor.matmul(ps, lhsT=kT_sl(kt, et), rhs=qT_sb[:, et, qr],
                                 start=(et == 0), stop=(et == ET - 1))
            nc.scalar.activation(out=p_sb[:, kt, qr], in_=ps,
                                 func=mybir.ActivationFunctionType.Exp,
                                 bias=shift_sb[:, 0:1], scale=float(SCALE))
            nc.vector.tensor_scalar_add(p8_sb[:, kt, qr], p_sb[:, kt, qr], 0.0)
    # denominators hoisted out of the qs loop: by the time qs=0's run, the
    # trailing DVE p8 copies of its last kt tiles finished a full scores
    # pass ago, so the PE never waits on the ACT->DVE chain
    for qs in range(SQ // NT):
        qr = slice(qs * NT, (qs + 1) * NT)
        ps_dt = ps_main.tile([P, NT], FP32, name="ps")
        for kt in range(0, KT, 2):
            nc.tensor.matmul(ps_dt, lhsT=ones8,
                             rhs=p8_sb[:, kt:kt + 2, qr],
                             start=(kt == 0), stop=(kt == KT - 2),
                             perf_mode=mybir.MatmulPerfMode.DoubleRow)
        nc.vector.reciprocal(out=recip_sb[:, qr], in_=ps_dt[0:1, :])
    recip_dram = dram.tile([1, SQ], FP32)
    nc.gpsimd.dma_start(out=recip_dram, in_=recip_sb)
    recip_bc = singles.tile([P, SQ], FP32)
    nc.gpsimd.dma_start(out=recip_bc, in_=recip_dram[0, :].partition_broadcast(P))

    # ---- O^T = V^T P, normalize, out (bf16; host upcasts)
    for ft in range(ET):
        fr = slice(ft * P, (ft + 1) * P)
        for qs in range(SQ // NT):
            qr = slice(qs * NT, (qs + 1) * NT)
            ps = ps_main.tile([P, NT], FP32)
            for kt in range(KT):
                nc.tensor.matmul(ps, lhsT=v_sb[:, kt, fr], rhs=p_sb[:, kt, qr],
                                 start=(kt == 0), stop=(kt == KT - 1))
            ot = outp.tile([P, NT], BF16)
            nc.vector.tensor_mul(ot, ps, recip_bc[:, qr])
            nc.sync.dma_start(out=outT[fr, qr], in_=ot)


def build_program():
    nc = bacc.Bacc("TRN2", target_bir_lowering=False, debug=False,
                   num_devices=N_CORES)
    io = {
        "xo": nc.dram_tensor("xo", [E, SQ], BF16, kind="ExternalInput").ap(),
        "wqT": nc.dram_tensor("wqT", [E, E], BF16, kind="ExternalInput").ap(),
        "wkT": nc.dram_tensor("wkT", [E, E], BF16, kind="ExternalInput").ap(),
        "wvT": nc.dram_tensor("wvT", [E, E], BF16, kind="ExternalInput").ap(),
        "bq": nc.dram_tensor("bq", [E], FP32, kind="ExternalInput").ap(),
        "bk": nc.dram_tensor("bk", [E], FP32, kind="ExternalInput").ap(),
        "bv": nc.dram_tensor("bv", [E], FP32, kind="ExternalInput").ap(),
        "outT": nc.dram_tensor("outT", [E, SQ], BF16, kind="ExternalOutput").ap(),
    }
    from contextlib import ExitStack
    with tile.TileContext(nc) as tc:
        with ExitStack() as ctx:
            build_kernel(ctx, tc, io)
    nc.compile()
    return nc


def make_in_maps(x, wq_w, wq_b, wk_w, wk_b, wv_w, wv_b):
    bf = ml_dtypes.bfloat16
    xT_all = np.ascontiguousarray(np.transpose(np.asarray(x, np.float32),
                                               (0, 2, 1))).astype(bf)
    wqT = np.ascontiguousarray(np.asarray(wq_w, np.float32).T).astype(bf)
    wkT = np.ascontiguousarray(np.asarray(wk_w, np.float32).T).astype(bf)
    wvT = np.ascontiguousarray(np.asarray(wv_w, np.float32).T).astype(bf)
    bq = np.asarray(wq_b, np.float32)
    bk = np.asarray(wk_b, np.float32)
    bv = np.asarray(wv_b, np.float32)
    in_maps = []
    for c in range(N_CORES):
        b, h = divmod(c, 2)
        in_maps.append({
            "xo": np.ascontiguousarray(xT_all[b][:, h * SQ:(h + 1) * SQ]),
            "wqT": wqT, "wkT": wkT, "wvT": wvT,
            "bq": bq, "bk": bk, "bv": bv,
        })
    return in_maps


def assemble_out(results):
    out = np.empty((B, S, E), np.float32)
    for c in range(N_CORES):
        b, h = divmod(c, 2)
        out[b, h * SQ:(h + 1) * SQ, :] = results[c]["outT"].astype(np.float32).T
    return out


_NC_CACHE = None


def kernel(x, wq_w, wq_b, wk_w, wk_b, wv_w, wv_b):
    global _NC_CACHE
    if _NC_CACHE is None:
        _NC_CACHE = build_program()
    in_maps = make_in_maps(x, wq_w, wq_b, wk_w, wk_b, wv_w, wv_b)
    try:
        res = run_bass_kernel_spmd(_NC_CACHE, in_maps, list(range(N_CORES)))
    except Exception:
        # transient axon/device hiccups happen; one retry
        res = run_bass_kernel_spmd(_NC_CACHE, in_maps, list(range(N_CORES)))
    return assemble_out(res.results)
